# revision 43
# baseline (speedup 1.0000x reference)
r"""Boson-sampling probability |Perm(A)|^2 via Glynn's formula on 8 Trainium2 cores.

Math
----
perm(A) = 2^(1-n) * sum_{d in {-1,+1}^n} (prod_i d_i) * prod_j (sum_i d_i A_ij), n=20.
Terms for d and -d are equal, so enumerate d_19 = -1 only and double.

Sign-bit allocation for the remaining 19 bits:
  bits 0..8   -> free axis f (512)       [same on every core]
  bits 9..15  -> partition axis p (128)  [same on every core]
  bits 16..18 -> core c (8)

Row vector V_j(p,f,c) = Cp_c[p,j] + Cf[f,j] with
  Cp_c[p,j] = sum_{i=9..15} d_i(p) A[i,j] + sum_{i=16..18} d_i(c) A[i,j] - A[19,j]
  Cf[f,j]   = sum_{i=0..8} d_i(f) A[i,j]

Split the j-product into groups GA=0..6, GB=7..13, GC=14..19. Each group
product expands over subsets T of the group:
  PG[p,f] = sum_T (prod_{j in T} Cp[p,j]) * (prod_{j in G\T} Cf[f,j])
a bilinear form of rank 2^|G| -> computed on TensorE as fp32 matmuls with
PSUM accumulation (contraction over 2*2^|G| re/im-expanded rows). VectorE
combines M = PC*PA (complex) while group B's matmuls still run, then four
fused multiply-reduce ops against PB in PSUM produce per-partition partials;
the (128,4) per-core partials are summed on host in float64.

DMA: all loads on the two HWDGE rings (SWDGE's ~2us completion latency
gated the whole PE pipeline in v1). SP ring carries C then B with
cumulative thresholds on one semaphore (same-ring completions are ordered);
ACT ring carries A.
"""

import numpy as np

N = 20
N_CORES = 8
F = 512           # free size (bits 0..8)
P = 128           # partitions (bits 9..15)
GA = list(range(0, 7))
GB = list(range(7, 14))
GC = list(range(14, 20))
W = 2 * P + F     # per-chunk packed width: [lhsT_re | lhsT_im | V]

_PROGRAM_CACHE = {}


def _signs(count, nbits):
    v = np.arange(count, dtype=np.int64)[:, None]
    return (((v >> np.arange(nbits)) & 1) * 2.0 - 1.0)  # (count, nbits) float64


def _subset_prods(C):
    """C: (nvals, g) complex128 -> (2^g, nvals); row T = prod_{k: bit k of T} C[:, k]."""
    out = np.ones((1, C.shape[0]), np.complex128)
    for k in range(C.shape[1]):
        out = np.concatenate([out, out * C[None, :, k]], axis=0)
    return out


def _pack_group(U, V):
    """Interleave re/im rows for the paired-contraction matmul layout.

    One shared V table streams through two matmuls; the re/im arithmetic is
    carried by two lhsT variants (contraction rows m = 2T + c):
      vtab[2T]   = Re V[T],  vtab[2T+1]   = Im V[T]
      lhs_re[2T] = Re U[T],  lhs_re[2T+1] = -Im U[T]   (-> PG_re)
      lhs_im[2T] = Im U[T],  lhs_im[2T+1] =  Re U[T]   (-> PG_im)
    """
    nT = U.shape[0]
    lre = np.empty((2 * nT, U.shape[1]), np.float32)
    lre[0::2] = U.real
    lre[1::2] = -U.imag
    lim = np.empty((2 * nT, U.shape[1]), np.float32)
    lim[0::2] = U.imag
    lim[1::2] = U.real
    vtab = np.empty((2 * nT, V.shape[1]), np.float32)
    vtab[0::2] = V.real
    vtab[1::2] = V.imag
    return lre, lim, vtab


def _build_core_tables(A, core):
    """Host tables for one core. A: (20,20) complex128.

    Each group packs to (128, nch*W): chunk k (contraction rows 128k..) at
    columns [k*W, (k+1)*W), laid out [lhsT_re | lhsT_im | V] per chunk.
    """
    f_signs = _signs(F, 9)
    p_signs = _signs(P, 7)
    c_signs = _signs(N_CORES, 3)
    par_f = np.prod(f_signs, axis=1)
    par_p = np.prod(p_signs, axis=1)
    par_c = np.prod(c_signs[core])

    Cf = f_signs @ A[0:9, :]                                         # (512, 20)
    Cp = p_signs @ A[9:16, :] + (c_signs[core] @ A[16:19, :] - A[19, :])[None, :]

    out = {}
    for name, G in (("A", GA), ("B", GB), ("C", GC)):
        U = _subset_prods(Cp[:, G])          # (2^g, 128)
        VV = _subset_prods(Cf[:, G])         # (2^g, 512)
        V = VV[::-1]                         # complement subset: T -> 2^g-1-T
        if name == "A":
            # fold full parity: par_p(p) * par_f(f) * par_c * (-1 for d19)
            U = U * (par_p[None, :] * (-par_c))
            V = V * par_f[None, :]
        lre, lim, vtab = _pack_group(U, V)
        packed = np.concatenate([lre, lim, vtab], axis=1).astype(np.float16)
        nch = packed.shape[0] // 128
        out["tab" + name] = np.ascontiguousarray(
            np.concatenate([packed[k * 128:(k + 1) * 128] for k in range(nch)],
                           axis=1))           # (128, nch*W)
    return out


def _build_program():
    if "prog" in _PROGRAM_CACHE:
        return _PROGRAM_CACHE["prog"]

    from contextlib import ExitStack
    from concourse import bass, mybir

    f32 = mybir.dt.float32
    f16 = mybir.dt.float16
    # FP16 tables: native 1-cycle/row PE path and half the DMA bytes of
    # fp32. Only the table values are rounded (2^-11); products accumulate
    # exactly in fp32 PSUM -> measured ~4e-4 end-to-end error.
    mm_dt = mybir.dt.float16
    mul = mybir.AluOpType.mult
    nc = bass.Bass()

    # DRAM parameters (per-core data via in_maps; same program on all cores).
    groups = (("A", 2), ("B", 2), ("C", 1))
    dram = {}
    for g, nch in groups:
        dram[g] = nc.declare_dram_parameter("tab" + g, [128, nch * W], mm_dt,
                                            isOutput=False)
    out_dram = nc.declare_dram_parameter("out", [P, 4], f32, isOutput=True)

    es = ExitStack()
    with es:
        # one semaphore per load DMA: sem increments come from the 16 SDMA
        # engines independently, so a shared semaphore with cumulative
        # thresholds would count a mix of both transfers' increments.
        dma_c = es.enter_context(nc.semaphore("dma_c"))
        dma_a0 = es.enter_context(nc.semaphore("dma_a0"))
        dma_b = es.enter_context(nc.semaphore("dma_b"))
        pe_sem = es.enter_context(nc.semaphore("pe_sem"))
        act_sem = es.enter_context(nc.semaphore("act_sem"))
        dve_sem = es.enter_context(nc.semaphore("dve_sem"))

        sb = {}
        for g, nch in groups:
            sb[g] = es.enter_context(nc.sbuf_tensor("sb_tab" + g, [128, nch * W], mm_dt))
        names = ["sPCre", "sPCim", "sPAim", "sBre",
                 "t1", "t2", "t3", "t4", "U_", "W_", "P0", "P3", "scr", "scr2"]
        wt = {n: es.enter_context(nc.sbuf_tensor(n, [P, F], f16)) for n in names}
        out_t = es.enter_context(nc.sbuf_tensor("out_t", [P, 4], f32))
        dummy = es.enter_context(nc.sbuf_tensor("actwarm", [P, 2], f32))
        pg = {}
        for g in ("A", "B", "C"):
            for comp in ("re", "im"):
                pg[g + comp] = es.enter_context(
                    nc.psum_tensor("pg" + g + comp, [P, F], f32))

        def _sync_stream():
            sync = nc.sync
            # SP HWDGE ring: C (gates the first matmuls) then B (consumed
            # last); A rides the ACT ring concurrently. Measured variants:
            # all three serialized on one ring is +1us on A and B (each
            # transfer's completion receipt serializes behind the previous);
            # A+B on SP with C on ACT is +0.6us mean (C behind the ACT
            # ring's activation-table load loses its head start).
            sync.dma_start(sb["C"][:, :], dram["C"][:, :]).then_inc(dma_c, 16)
            sync.dma_start(sb["B"][:, :], dram["B"][:, :]).then_inc(dma_b, 16)

        def _act_stream():
            act = nc.scalar
            # A on the ACT HWDGE ring, concurrent with the SP ring. One
            # transfer: splitting per chunk was measured +1us mean (the
            # second chunk's completion receipt serializes behind the
            # first's on the same ring).
            act.dma_start(sb["A"][:, :], dram["A"][:, :]).then_inc(dma_a0, 16)
            # touch ACT before any gating wait so walrus's activation table
            # load happens during the DMA window, off the critical path
            # (reads uninitialized dummy SBUF -- the result is never used)
            act.copy(dummy[:, 1:2], dummy[:, 0:1])
            # PSUM->SBUF fp16 evictions (PC is evicted by the DVE, which is
            # idle earlier). PAre is NOT evicted: t1/t4 read it straight
            # from PSUM (1x, 658ns) which still beats waiting out a 690ns
            # eviction; PAim is evicted because two 2x-mode consumers
            # amortize the copy.
            act.wait_ge(pe_sem, 6)
            act.copy(wt["sPAim"][:, :], pg["Aim"][:, :]).then_inc(act_sem, 1)
            # scale by 1/16 during eviction: the triple products U_*sBre /
            # W_*sBre overflow fp16 (absmax ~1.8e5 > 65504) unscaled. The
            # host multiplies cols 0 and 3 back by 16.
            act.wait_ge(pe_sem, 8)
            act.mul(wt["sBre"][:, :], pg["Bre"][:, :], 0.0625).then_inc(act_sem, 1)
            # reduce the two DVE-produced products over f while the DVE runs
            # the remaining fused reduces: accum_out = sum(copy(P)).
            cp = mybir.ActivationFunctionType.Copy
            act.wait_ge(dve_sem, 8)
            act.activation(wt["scr2"][:, :], wt["P0"][:, :], cp,
                           accum_out=out_t[:, 0:1])
            act.wait_ge(dve_sem, 10)
            act.activation(wt["scr2"][:, :], wt["P3"][:, :], cp,
                           accum_out=out_t[:, 3:4])
            # store; the end-of-block engine drains cover DMA completion
            act.wait_ge(dve_sem, 12)
            act.dma_start(out_dram[:], out_t[:, :]).then_inc(dma_c, 16)

        def _pe_stream():
            pe = nc.tensor
            # group order C, A, B: C+A feed the DVE multiply chain early;
            # B (last) is consumed straight from PSUM by the final fused
            # reduces. pe_sem: PC done at 2, PAre 4, PAim 6, PBre 8, PBim 10.
            def mm(g, comp, k, nch, wait=None, thr=0):
                if wait is not None:
                    pe.wait_ge(wait, thr)
                lo = k * W + (0 if comp == "re" else P)
                pe.matmul(
                    pg[g + comp][:, :],
                    sb[g][:, lo:lo + P],
                    sb[g][:, k * W + 2 * P:k * W + 2 * P + F],
                    start=(k == 0),
                    stop=(k == nch - 1),
                ).then_inc(pe_sem, 1)
            mm("C", "re", 0, 1, dma_c, 16)
            mm("C", "im", 0, 1)
            mm("A", "re", 0, 2, dma_a0, 16)
            mm("A", "re", 1, 2)
            mm("A", "im", 0, 2)
            mm("A", "im", 1, 2)
            mm("B", "re", 0, 2, dma_b, 16)
            mm("B", "re", 1, 2)
            mm("B", "im", 0, 2)
            mm("B", "im", 1, 2)

        def _dve_stream():
            v = nc.vector
            # self-evict PC to fp16 SBUF (2x-mode copies) while A still loads
            v.wait_ge(pe_sem, 2)
            v.tensor_copy(wt["sPCre"][:, :], pg["Cre"][:, :]).then_inc(dve_sem, 1)
            v.tensor_copy(wt["sPCim"][:, :], pg["Cim"][:, :]).then_inc(dve_sem, 1)
            # M = PC*PA: the Are-side products read PAre straight from PSUM
            # (1x) the moment its matmuls finish; the Aim-side ones use the
            # ACT-evicted fp16 copy (2x). Engine order is program order, so
            # no same-engine self-waits needed.
            v.wait_ge(pe_sem, 4)
            v.tensor_mul(wt["t1"][:, :], wt["sPCre"][:, :], pg["Are"][:, :]).then_inc(dve_sem, 1)
            v.tensor_mul(wt["t4"][:, :], wt["sPCim"][:, :], pg["Are"][:, :]).then_inc(dve_sem, 1)
            v.wait_ge(act_sem, 1)
            v.tensor_mul(wt["t2"][:, :], wt["sPCim"][:, :], wt["sPAim"][:, :]).then_inc(dve_sem, 1)
            v.tensor_mul(wt["t3"][:, :], wt["sPCre"][:, :], wt["sPAim"][:, :]).then_inc(dve_sem, 1)
            v.tensor_sub(wt["U_"][:, :], wt["t1"][:, :], wt["t2"][:, :]).then_inc(dve_sem, 1)
            # out cols: 0 = sum U*PBre, 1 = sum W*PBim, 2 = sum U*PBim,
            # 3 = sum W*PBre ; host computes re = c0-c1, im = c2+c3.
            # The Bre-side products are cheap fp16 TTs (327ns) against the
            # ACT-evicted sBre; the ACT engine reduces them (cols 0 and 3)
            # while the DVE runs the two fused PSUM reduces (cols 2 and 1).
            # P0 goes before W_ so ACT's first reduce starts one op earlier.
            v.wait_ge(act_sem, 2)
            v.tensor_mul(wt["P0"][:, :], wt["U_"][:, :], wt["sBre"][:, :]).then_inc(dve_sem, 1)
            v.tensor_add(wt["W_"][:, :], wt["t3"][:, :], wt["t4"][:, :]).then_inc(dve_sem, 1)
            v.tensor_mul(wt["P3"][:, :], wt["W_"][:, :], wt["sBre"][:, :]).then_inc(dve_sem, 1)
            v.wait_ge(pe_sem, 10)
            v.scalar_tensor_tensor(
                wt["scr"][:, :], wt["U_"][:, :], 1.0, pg["Bim"][:, :],
                mul, mul, accum_out=out_t[:, 2:3]).then_inc(dve_sem, 1)
            v.scalar_tensor_tensor(
                wt["scr"][:, :], wt["W_"][:, :], 1.0, pg["Bim"][:, :],
                mul, mul, accum_out=out_t[:, 1:2]).then_inc(dve_sem, 1)

        _sync_stream()
        _act_stream()
        _pe_stream()
        _dve_stream()
        # no explicit epilogue: the NRT postamble quiesces DMA rings

    nc.finalize()
    _PROGRAM_CACHE["prog"] = nc
    return nc


def kernel(A_real, A_imag, _collect=None):
    from concourse.bass_utils import run_bass_kernel_spmd

    A = np.asarray(A_real, np.float64) + 1j * np.asarray(A_imag, np.float64)
    nc = _build_program()
    in_maps = [_build_core_tables(A, c) for c in range(N_CORES)]

    kwargs = dict(_collect or {})
    res = run_bass_kernel_spmd(nc, in_maps, core_ids=list(range(N_CORES)), **kwargs)
    if _collect is not None:
        _collect["results"] = res

    total = np.complex128(0)
    for r in res.results:
        o = np.asarray(r["out"], np.float64)
        # cols 0/3 were computed against sBre = PBre/16 (fp16 range): undo
        total += (16.0 * o[:, 0] - o[:, 1]).sum() + 1j * (o[:, 2] + 16.0 * o[:, 3]).sum()

    perm = total * 2.0 * (2.0 ** (1 - N))
    ans = (perm.conjugate() * perm).real
    return np.asarray(ans, np.float32)
